# revision 14
# baseline (speedup 1.0000x reference)
"""Trainium2 Bass kernel for the IMU preintegration module.

Full inputs in, full outputs out; internally data-parallel over 8 NeuronCores
(512 batch rows per core).

Math: the scan's per-row state composes associatively as (R, b, d) with
    b = sum_t R_{1..t} a_t,   d = sum_t (S-1-t) R_{1..t} a_t   (raw units;
physical factors of dt are folded into scalars).  Composition of two units
A,B of n steps each:
    R = RA RB,  b = bA + RA bB,  d = dA + n bA + RA dB.
Per-step rotations are tiny (|w| dt ~ 0.01 rad), so:
  L0: groups of n0=4 steps reduce to (theta, b, rho) with first-order
      rotation:  theta = sum w (+ BCH pair term), b = V + (dt/4)(theta x u
      + p x aw)  [u = ramp-weighted a-sum, p = Haar mode of w, aw its
      weight], rho = 3a0+2a1+a2.
  SA: small-angle composition up to 64-step units:
      theta' = tA+tB + (dt/2) tA x tB,  b' = bA+bB + dt (tA x bB),
      rho' = rA + n bA + rB.
  QT: convert theta -> quaternion (2-3 term polys in h = |theta dt/2|^2),
      exact quaternion binary tree for the remaining 5 levels.
Validated in numpy against the jax reference: rel err ~3.7e-3 (gate 2e-2).

Perf notes (hardware-measured): DVE fp32 ops run 1 elem/cycle at read
stride <= 8B, ~1.5x slower at 16-48B, ~2x at 96B.  So ScalarE (otherwise
idle) deinterleaves the accel channels into dense tiles; DVE reads only
dense / stride-2 data except the two w pair-sum ops.  GpSimd takes the
independent V-chain + one cross per slab.  SBUF: 8 rotating 6KB plane
buffers (Q1-Q8) hold all group-level 3-vectors via explicit reuse.
"""

import math
import os
import numpy as np

import concourse.mybir as mybir
from concourse import bass, bacc
from concourse.tile import TileContext

F32 = mybir.dt.float32
BF16 = mybir.dt.bfloat16
OP = mybir.AluOpType
AF = mybir.ActivationFunctionType

# problem constants (hardcoded per harness contract)
B_FULL = 4096
S_FULL = 2048
C = 6
N_CORES = 8
R = B_FULL // N_CORES          # rows per core = 512
DT = float(np.float32(1.0 / 200.0))

QSGN = [(1, -1, -1, -1), (1, 1, 1, -1), (1, -1, 1, 1), (1, 1, -1, 1)]
QIDX = [(0, 1, 2, 3), (1, 0, 3, 2), (2, 3, 0, 1), (3, 2, 1, 0)]


def build_nc(rows=R, s_len=S_FULL, s_chunk=512):
    nc = bacc.Bacc(None, target_bir_lowering=False, debug=False)
    g = rows // 128                    # 4 rows per partition
    n_slabs = s_len // s_chunk         # 4
    G = s_chunk // 4                   # 128 4-step groups per slab
    U8 = s_chunk // 8                  # 64 8-step units per slab
    U16 = s_chunk // 16                # 32 16-step units per slab
    NU16 = s_len // 16                 # 128 16-step units per row
    x = nc.dram_tensor("x", [rows, s_len, C], F32, kind="ExternalInput")
    out = nc.dram_tensor("out", [rows, 7], F32, kind="ExternalOutput")
    xv = x.rearrange("(g p) s c -> g p s c", g=g)

    V = nc.vector
    Gp = nc.gpsimd
    Sc = nc.scalar

    def comps(t, n=3):
        return [t[:, :, ci, :] for ci in range(n)]

    def cross_into(eng, dst, u, v, tmps):
        """dst = u x v per-component (lists of [128,g,U] views). 9 ops."""
        for i in range(3):
            i1, i2 = (i + 1) % 3, (i + 2) % 3
            eng.tensor_tensor(out=tmps[0], in0=u[i1], in1=v[i2], op=OP.mult)
            eng.tensor_tensor(out=tmps[1], in0=u[i2], in1=v[i1], op=OP.mult)
            eng.tensor_tensor(out=dst[i], in0=tmps[0], in1=tmps[1],
                              op=OP.subtract)

    def qmul_into(nq, q1, q2, tmps):
        """nq = q1 (x) q2 elementwise (lists of 4 views). 28 DVE ops."""
        for comp in range(4):
            acc = nq[comp]
            V.tensor_tensor(out=acc, in0=q1[0], in1=q2[QIDX[comp][0]],
                            op=OP.mult)
            for t in range(1, 4):
                tmp = tmps[t % 2]
                V.tensor_tensor(out=tmp, in0=q1[t], in1=q2[QIDX[comp][t]],
                                op=OP.mult)
                V.tensor_tensor(out=acc, in0=acc, in1=tmp,
                                op=OP.add if QSGN[comp][t] > 0 else OP.subtract)

    with TileContext(nc) as tc:
        with (
            tc.tile_pool(name="slab", bufs=2) as slab_pool,
            tc.tile_pool(name="deint", bufs=1) as deint_pool,
            tc.tile_pool(name="plane", bufs=1) as plane_pool,
            tc.tile_pool(name="xtmp", bufs=2) as xtmp_pool,
            tc.tile_pool(name="acc", bufs=1) as acc_pool,
            tc.tile_pool(name="tree", bufs=1) as tree_pool,
        ):
            pshape = [128, g, 3, G]

            def plane(buf, nm):
                return plane_pool.tile(pshape, F32, tag=f"q{buf}", name=nm)

            def vtmp(n=None):
                return [xtmp_pool.tile([128, g, G], F32, tag="vt",
                                       name="vt")[:, :, 0:(n or G)]
                        for _ in range(2)]

            def gtmp(n=None):
                return [xtmp_pool.tile([128, g, G], F32, tag="gt",
                                       name="gt")[:, :, 0:(n or G)]
                        for _ in range(2)]

            # persistent accumulators: 16-step units across all slabs
            th16 = acc_pool.tile([128, g, 3, NU16], F32, tag="t16",
                                 name="t16")
            b16 = acc_pool.tile([128, g, 3, NU16], F32, tag="bb16",
                                name="bb16")
            r16 = acc_pool.tile([128, g, 3, NU16], F32, tag="r16", name="r16")

            def sa_fold(oth, cs, scal):
                # oth += scal * cs, per component (stt needs <=2 free dims)
                for ci in range(3):
                    V.scalar_tensor_tensor(out=oth[:, :, ci, :],
                                           in0=cs[:, :, ci, :], scalar=scal,
                                           in1=oth[:, :, ci, :],
                                           op0=OP.mult, op1=OP.add)

            def sa_level(thI, bI, rI, n_steps, oth, ob, orr, cs, cs2, tmp,
                         radd_eng):
                thAc = [thI[:, :, ci, 0::2] for ci in range(3)]
                thBc = [thI[:, :, ci, 1::2] for ci in range(3)]
                bBc = [bI[:, :, ci, 1::2] for ci in range(3)]
                cross_into(V, comps(cs), thAc, thBc, tmp)
                V.tensor_tensor(out=oth, in0=thI[:, :, :, 0::2],
                                in1=thI[:, :, :, 1::2], op=OP.add)
                sa_fold(oth, cs, DT / 2.0)
                cross_into(V, comps(cs2), thAc, bBc, tmp)
                V.tensor_tensor(out=ob, in0=bI[:, :, :, 0::2],
                                in1=bI[:, :, :, 1::2], op=OP.add)
                sa_fold(ob, cs2, DT)
                radd_eng.tensor_tensor(out=orr, in0=rI[:, :, :, 0::2],
                                       in1=rI[:, :, :, 1::2], op=OP.add)
                for ci in range(3):
                    V.scalar_tensor_tensor(out=orr[:, :, ci, :],
                                           in0=bI[:, :, ci, 0::2],
                                           scalar=float(n_steps),
                                           in1=orr[:, :, ci, :],
                                           op0=OP.mult, op1=OP.add)

            for si in range(n_slabs):
                slab = slab_pool.tile([128, g, s_chunk, C], F32, tag="slab",
                                      name=f"slab{si}")
                for gi in range(g):
                    eng = nc.sync if gi % 2 == 0 else nc.gpsimd
                    eng.dma_start(
                        out=slab[:, gi],
                        in_=xv[gi, :, si * s_chunk:(si + 1) * s_chunk, :],
                    )

                # ScalarE: deinterleave accel channels into dense tiles.
                # Order At1, At0 first: the DVE u-chain reads them first.
                At = [deint_pool.tile(pshape, F32, tag=f"at{i}",
                                      name=f"at{i}") for i in range(4)]
                for i in (1, 0, 2, 3):
                    for ci in range(3):
                        Sc.copy(At[i][:, :, ci, :], slab[:, :, i::4, 3 + ci])

                # DVE: w pair sums (strided slab reads), theta (dense),
                # p directly in bf16 (only feeds the bf16 correction cross)
                W = [slab[:, :, i::4, 0:3].transpose([0, 1, 3, 2])
                     for i in range(4)]
                s01 = plane(1, "s01")
                s23 = plane(2, "s23")
                th4 = plane(3, "th4")
                q4h = plane(4, "q4h")[:].bitcast(BF16)   # two bf16 planes
                p4h = q4h[:, :, :, 0:G]
                u4h = q4h[:, :, :, G:2 * G]
                q5h = plane(5, "q5h")[:].bitcast(BF16)
                th4h = q5h[:, :, :, 0:G]
                awh = q5h[:, :, :, G:2 * G]
                V.tensor_tensor(out=s01[:], in0=W[0], in1=W[1], op=OP.add)
                V.tensor_tensor(out=s23[:], in0=W[2], in1=W[3], op=OP.add)
                V.tensor_tensor(out=th4[:], in0=s01[:], in1=s23[:], op=OP.add)
                V.tensor_tensor(out=p4h, in0=s01[:], in1=s23[:],
                                op=OP.subtract)
                Sc.copy(th4h, th4[:])

                # GpSimd: BCH cross c3 = s01 x s23 (its only compute),
                # bf16 output into a dedicated 3KB tile
                c3h = plane_pool.tile([128, g, 3, G], BF16, tag="c3h",
                                      name="c3h")
                gt = [xtmp_pool.tile([128, g, G], F32, tag="gt",
                                     name="gt")[:].bitcast(BF16)[:, :, 0:G]
                      for _ in range(2)]
                cross_into(Gp, comps(c3h), comps(s01), comps(s23), gt)
                vv = plane(8, "vv")
                V.tensor_tensor(out=vv[:].opt(), in0=At[0][:].opt(),
                                in1=At[1][:].opt(), op=OP.add)
                V.tensor_tensor(out=vv[:].opt(), in0=vv[:].opt(),
                                in1=At[2][:].opt(), op=OP.add)
                V.tensor_tensor(out=vv[:].opt(), in0=vv[:].opt(),
                                in1=At[3][:].opt(), op=OP.add)

                # DVE: u-chain / aw / rho on dense At (flattened stt)
                t0 = plane(6, "t0")
                V.scalar_tensor_tensor(out=t0[:].opt(), in0=At[1][:].opt(),
                                       scalar=2.0, in1=At[0][:].opt(),
                                       op0=OP.mult, op1=OP.add)
                u4f = plane(7, "u4f")
                V.scalar_tensor_tensor(out=u4f[:].opt(), in0=At[2][:].opt(),
                                       scalar=3.0, in1=t0[:].opt(),
                                       op0=OP.mult, op1=OP.add)
                V.scalar_tensor_tensor(out=u4h.opt(), in0=At[3][:].opt(),
                                       scalar=4.0, in1=u4f[:].opt(),
                                       op0=OP.mult, op1=OP.add)
                # bf16 correction crosses (2x DVE mode)
                q2h = plane(2, "q2h")       # reuse s23 (dead after th/p)
                q2hb = q2h[:].bitcast(BF16)
                c1h = q2hb[:, :, :, 0:G]
                c2h = q2hb[:, :, :, G:2 * G]
                btmp = [xtmp_pool.tile([128, g, G], F32, tag="vt",
                                       name="vt")[:].bitcast(BF16)[:, :, 0:G]
                        for _ in range(2)]
                cross_into(V, comps(c1h), comps(th4h), comps(u4h), btmp)
                # aw = t0 + a2 (in place on t0)
                V.tensor_tensor(out=t0[:], in0=t0[:], in1=At[2][:], op=OP.add)
                aw = t0
                Sc.copy(awh, aw[:])
                # c2 = p4 x aw (bf16)
                cross_into(V, comps(c2h), comps(p4h), comps(awh), btmp)
                # rho = 2 a0 + aw  (u4f dead after its cast -> reuse Q7)
                rho4 = plane(7, "rho4")
                V.scalar_tensor_tensor(out=rho4[:].opt(), in0=At[0][:].opt(),
                                       scalar=2.0, in1=aw[:].opt(),
                                       op0=OP.mult, op1=OP.add)

                # folds: theta += (dt/2) c3 ; b = V + (dt/4)(c1+c2)
                V.scalar_tensor_tensor(out=th4[:].opt(), in0=c3h[:].opt(),
                                       scalar=DT / 2.0, in1=th4[:].opt(),
                                       op0=OP.mult, op1=OP.add)
                V.tensor_tensor(out=c1h, in0=c1h, in1=c2h, op=OP.add)
                V.scalar_tensor_tensor(out=vv[:].opt(), in0=c1h.opt(),
                                       scalar=DT / 4.0, in1=vv[:].opt(),
                                       op0=OP.mult, op1=OP.add)
                b4 = vv

                # ---- SA1: n=4 -> 8 ----
                th8 = plane(1, "th8")[:, :, :, 0:U8]   # s01 dead
                b8v = plane(2, "b8")[:, :, :, 0:U8]    # c1h/c2h dead
                r8v = plane(5, "r8")[:, :, :, 0:U8]    # th4h/awh dead
                csa = plane(4, "csa")[:, :, :, 0:U8]   # p4h/u4h dead
                csb = plane(6, "csb")[:, :, :, 0:U8]   # aw dead
                sa_level(th4[:], b4[:], rho4[:], 4, th8, b8v, r8v,
                         csa, csb, vtmp(U8), Gp)

                # ---- SA2: n=8 -> 16, into accumulators ----
                oth = th16[:, :, :, si * U16:(si + 1) * U16]
                ob = b16[:, :, :, si * U16:(si + 1) * U16]
                orr = r16[:, :, :, si * U16:(si + 1) * U16]
                csa2 = plane(4, "csa2")[:, :, :, 0:U16]
                csb2 = plane(6, "csb2")[:, :, :, 0:U16]
                sa_level(th8, b8v, r8v, 8, oth, ob, orr,
                         csa2, csb2, vtmp(U16), Gp)

            # ---- batched SA levels: 16->32->64 ----
            th32 = plane(1, "th32")[:, :, :, 0:64]
            b32 = plane(2, "b32")[:, :, :, 0:64]
            r32 = plane(5, "r32")[:, :, :, 0:64]
            cs_a = plane(4, "cs_a")[:, :, :, 0:64]
            cs_b = plane(6, "cs_b")[:, :, :, 0:64]
            sa_level(th16[:], b16[:], r16[:], 16, th32, b32, r32,
                     cs_a, cs_b, vtmp(64), Gp)
            th64 = plane(3, "th64")[:, :, :, 0:32]
            b64 = plane(8, "b64")[:, :, :, 0:32]
            r64 = plane(7, "r64")[:, :, :, 0:32]
            cs_a2 = plane(4, "cs_a2")[:, :, :, 0:32]
            cs_b2 = plane(6, "cs_b2")[:, :, :, 0:32]
            sa_level(th32, b32, r32, 32, th64, b64, r64,
                     cs_a2, cs_b2, vtmp(32), Gp)

            # ---- convert 64-step units to quaternions ----
            NU = 32
            K2 = (DT / 2.0) ** 2
            h2 = tree_pool.tile([128, g, NU], F32, tag="h2", name="h2")
            hy = tree_pool.tile([128, g, NU], F32, tag="hy", name="hy")
            hz = tree_pool.tile([128, g, NU], F32, tag="hz", name="hz")
            q64 = tree_pool.tile([128, g, 4, NU], F32, tag="q64", name="q64")
            V.tensor_tensor(out=h2[:], in0=th64[:, :, 0, :],
                            in1=th64[:, :, 0, :], op=OP.mult)
            for ci in (1, 2):
                V.tensor_tensor(out=hy[:], in0=th64[:, :, ci, :],
                                in1=th64[:, :, ci, :], op=OP.mult)
                V.tensor_tensor(out=h2[:], in0=h2[:], in1=hy[:], op=OP.add)
            # qw = 1 - (k/2) h2 + (k^2/24) h2^2
            V.scalar_tensor_tensor(out=hy[:], in0=h2[:],
                                   scalar=K2 * K2 / 24.0,
                                   in1=h2[:], op0=OP.mult, op1=OP.mult)
            V.scalar_tensor_tensor(out=hz[:], in0=h2[:], scalar=-K2 / 2.0,
                                   in1=hy[:], op0=OP.mult, op1=OP.add)
            Sc.activation(q64[:, :, 0, :], hz[:], AF.Identity, bias=1.0)
            # qv = (dt/2)(1 - (k/6) h2 + (k^2/120) h2^2) * theta
            V.scalar_tensor_tensor(out=hy[:], in0=h2[:],
                                   scalar=K2 * K2 / 120.0,
                                   in1=h2[:], op0=OP.mult, op1=OP.mult)
            V.scalar_tensor_tensor(out=hz[:], in0=h2[:], scalar=-K2 / 6.0,
                                   in1=hy[:], op0=OP.mult, op1=OP.add)
            Sc.activation(hz[:], hz[:], AF.Copy, scale=DT / 2.0,
                          bias=DT / 2.0)
            for ci in range(3):
                V.tensor_tensor(out=q64[:, :, 1 + ci, :],
                                in0=th64[:, :, ci, :], in1=hz[:], op=OP.mult)

            # ---- quaternion binary tree over 32 units (5 levels) ----
            cur_q = [q64[:, :, ci, :] for ci in range(4)]
            cur_b = [b64[:, :, ci, :] for ci in range(3)]
            cur_d = [r64[:, :, ci, :] for ci in range(3)]
            n2 = 64.0
            n_lvl = int(math.log2(NU))
            for lvl in range(1, n_lvl + 1):
                n = NU >> lvl
                lshp = [128, g, n]
                q1 = [t[:, :, 0::2] for t in cur_q]
                q2 = [t[:, :, 1::2] for t in cur_q]
                b1 = [t[:, :, 0::2] for t in cur_b]
                b2 = [t[:, :, 1::2] for t in cur_b]
                d1 = [t[:, :, 0::2] for t in cur_d]
                d2 = [t[:, :, 1::2] for t in cur_d]

                nq = [tree_pool.tile(lshp, F32, tag=f"tq{lvl}_{i}",
                                     name=f"tq{lvl}_{i}")[:] for i in range(4)]
                nb = [tree_pool.tile(lshp, F32, tag=f"tb{lvl}_{i}",
                                     name=f"tb{lvl}_{i}")[:] for i in range(3)]
                nd = [tree_pool.tile(lshp, F32, tag=f"td{lvl}_{i}",
                                     name=f"td{lvl}_{i}")[:] for i in range(3)]
                tc_c = [tree_pool.tile(lshp, F32, tag=f"tc{i}",
                                       name=f"tc{i}")[:] for i in range(3)]
                tc_w = [tree_pool.tile(lshp, F32, tag=f"tw{i}",
                                       name=f"tw{i}")[:] for i in range(3)]
                tm = [tree_pool.tile(lshp, F32, tag=f"tm{i}",
                                     name=f"tm{i}")[:] for i in range(2)]

                qmul_into(nq, q1, q2, tm)
                qv1 = q1[1:]

                # nd = d1 + n2*b1 + rot(q1, d2)
                cross_into(V, tc_c, qv1, d2, tm)
                for i in range(3):
                    V.tensor_tensor(out=tc_w[i], in0=q1[0], in1=d2[i],
                                    op=OP.mult)
                for i in range(3):
                    V.tensor_tensor(out=tc_c[i], in0=tc_c[i], in1=tc_w[i],
                                    op=OP.add)
                cross_into(V, tc_w, qv1, tc_c, tm)
                for i in range(3):
                    V.scalar_tensor_tensor(out=nd[i], in0=b1[i], scalar=n2,
                                           in1=d1[i], op0=OP.mult, op1=OP.add)
                for i in range(3):
                    Gp.tensor_tensor(out=nd[i], in0=nd[i], in1=d2[i],
                                     op=OP.add)
                for i in range(3):
                    V.scalar_tensor_tensor(out=nd[i], in0=tc_w[i], scalar=2.0,
                                           in1=nd[i], op0=OP.mult, op1=OP.add)

                # nb = b1 + rot(q1, b2)
                cross_into(V, tc_c, qv1, b2, tm)
                for i in range(3):
                    V.tensor_tensor(out=tc_w[i], in0=q1[0], in1=b2[i],
                                    op=OP.mult)
                for i in range(3):
                    V.tensor_tensor(out=tc_c[i], in0=tc_c[i], in1=tc_w[i],
                                    op=OP.add)
                cross_into(V, tc_w, qv1, tc_c, tm)
                for i in range(3):
                    Gp.tensor_tensor(out=nb[i], in0=b1[i], in1=b2[i],
                                     op=OP.add)
                for i in range(3):
                    V.scalar_tensor_tensor(out=nb[i], in0=tc_w[i], scalar=2.0,
                                           in1=nb[i], op0=OP.mult, op1=OP.add)

                cur_q, cur_b, cur_d = nq, nb, nd
                n2 *= 2.0

            # ---- finalize ----
            out_t = tree_pool.tile([128, g, 7], F32, tag="outt",
                                   name="outt")[:]
            tf = tree_pool.tile([128, g, 1], F32, tag="tf", name="tf")[:]
            for i in range(3):
                V.scalar_tensor_tensor(out=tf, in0=cur_b[i], scalar=1.5,
                                       in1=cur_d[i], op0=OP.mult, op1=OP.add)
                V.tensor_scalar(out=out_t[:, :, i:i + 1], in0=tf,
                                scalar1=DT * DT, scalar2=None, op0=OP.mult)
            sg = tree_pool.tile([128, g, 1], F32, tag="sg", name="sg")[:]
            V.tensor_scalar(out=sg, in0=cur_q[0], scalar1=0.0, scalar2=2.0,
                            op0=OP.is_ge, op1=OP.mult)
            V.tensor_scalar(out=sg, in0=sg, scalar1=-1.0, scalar2=None,
                            op0=OP.add)
            for i in range(4):
                V.tensor_tensor(out=out_t[:, :, 3 + i:4 + i], in0=cur_q[i],
                                in1=sg, op=OP.mult)

            ov = out.rearrange("(g p) c -> g p c", g=g)
            for gi in range(g):
                nc.sync.dma_start(out=ov[gi], in_=out_t[:, gi, :])

    nc.compile()
    return nc


_NC_CACHE = {}
LAST_RESULTS = None


def _ensure_profiling_hooks():
    """Best-effort: provide the antenv.axon_hooks shim + skip S3 upload so
    trace=True works in this stripped container. No-op on failure."""
    import sys
    import types
    try:
        if "antenv.axon_hooks" not in sys.modules:
            from trn_agent_boot.trn_boot import _ntff_profile_via_ctypes
            hook = _ntff_profile_via_ctypes("/opt/axon/libaxon_pjrt.so")
            mod = types.ModuleType("antenv.axon_hooks")
            mod._hook = hook
            mod.get_axon_ntff_profile_hook = lambda: mod._hook
            mod.set_axon_ntff_profile_hook = lambda h: setattr(mod, "_hook", h)
            sys.modules["antenv.axon_hooks"] = mod
        import concourse.bass_utils as bu
        bu.upload_artifacts = lambda tmpdir: tmpdir
    except Exception as e:  # pragma: no cover
        print(f"profiling hook setup failed ({e}); tracing may be skipped")


def kernel(input_seq: np.ndarray) -> np.ndarray:
    from concourse.bass_utils import run_bass_kernel_spmd

    global LAST_RESULTS
    input_seq = np.ascontiguousarray(np.asarray(input_seq, dtype=np.float32))
    assert input_seq.shape == (B_FULL, S_FULL, C), input_seq.shape

    if "nc" not in _NC_CACHE:
        _NC_CACHE["nc"] = build_nc()
    nc = _NC_CACHE["nc"]

    in_maps = [{"x": input_seq[i * R:(i + 1) * R]} for i in range(N_CORES)]
    trace = os.environ.get("BASS_KERNEL_TRACE", "0") == "1"
    if trace:
        _ensure_profiling_hooks()
    res = run_bass_kernel_spmd(nc, in_maps, core_ids=list(range(N_CORES)),
                               trace=trace)
    LAST_RESULTS = res
    return np.concatenate([r["out"] for r in res.results], axis=0)


# revision 15
# speedup vs baseline: 1.0427x; 1.0427x over previous
"""Trainium2 Bass kernel for the IMU preintegration module.

Full inputs in, full outputs out; internally data-parallel over 8 NeuronCores
(512 batch rows per core).

Math: the scan's per-row state composes associatively as (R, b, d) with
    b = sum_t R_{1..t} a_t,   d = sum_t (S-1-t) R_{1..t} a_t   (raw units;
physical factors of dt are folded into scalars).  Composition of two units
A,B of n steps each:
    R = RA RB,  b = bA + RA bB,  d = dA + n bA + RA dB.
Per-step rotations are tiny (|w| dt ~ 0.01 rad), so:
  L0: groups of n0=4 steps reduce to (theta, b, rho) with first-order
      rotation:  theta = sum w (+ BCH pair term), b = V + (dt/4)(theta x u
      + p x aw)  [u = ramp-weighted a-sum, p = Haar mode of w, aw its
      weight], rho = 3a0+2a1+a2.
  SA: small-angle composition up to 64-step units:
      theta' = tA+tB + (dt/2) tA x tB,  b' = bA+bB + dt (tA x bB),
      rho' = rA + n bA + rB.
  QT: convert theta -> quaternion (2-3 term polys in h = |theta dt/2|^2),
      exact quaternion binary tree for the remaining 5 levels.
Validated in numpy against the jax reference: rel err ~3.7e-3 (gate 2e-2).

Perf notes (hardware-measured): DVE fp32 ops run 1 elem/cycle at read
stride <= 8B, ~1.5x slower at 16-48B, ~2x at 96B.  So ScalarE (otherwise
idle) deinterleaves the accel channels into dense tiles; DVE reads only
dense / stride-2 data except the two w pair-sum ops.  GpSimd takes the
independent V-chain + one cross per slab.  SBUF: 8 rotating 6KB plane
buffers (Q1-Q8) hold all group-level 3-vectors via explicit reuse.
"""

import math
import os
import numpy as np

import concourse.mybir as mybir
from concourse import bass, bacc
from concourse.tile import TileContext

F32 = mybir.dt.float32
BF16 = mybir.dt.bfloat16
OP = mybir.AluOpType
AF = mybir.ActivationFunctionType

# problem constants (hardcoded per harness contract)
B_FULL = 4096
S_FULL = 2048
C = 6
N_CORES = 8
R = B_FULL // N_CORES          # rows per core = 512
DT = float(np.float32(1.0 / 200.0))

QSGN = [(1, -1, -1, -1), (1, 1, 1, -1), (1, -1, 1, 1), (1, 1, -1, 1)]
QIDX = [(0, 1, 2, 3), (1, 0, 3, 2), (2, 3, 0, 1), (3, 2, 1, 0)]


def build_nc(rows=R, s_len=S_FULL, s_chunk=512):
    nc = bacc.Bacc(None, target_bir_lowering=False, debug=False)
    g = rows // 128                    # 4 rows per partition
    n_slabs = s_len // s_chunk         # 4
    G = s_chunk // 4                   # 128 4-step groups per slab
    U8 = s_chunk // 8                  # 64 8-step units per slab
    U16 = s_chunk // 16                # 32 16-step units per slab
    NU16 = s_len // 16                 # 128 16-step units per row
    x = nc.dram_tensor("x", [rows, s_len, C], F32, kind="ExternalInput")
    out = nc.dram_tensor("out", [rows, 7], F32, kind="ExternalOutput")
    xv = x.rearrange("(g p) s c -> g p s c", g=g)

    V = nc.vector
    Gp = nc.gpsimd
    Sc = nc.scalar

    def comps(t, n=3):
        return [t[:, :, ci, :] for ci in range(n)]

    def cross_into(eng, dst, u, v, tmps):
        """dst = u x v per-component (lists of [128,g,U] views). 9 ops."""
        for i in range(3):
            i1, i2 = (i + 1) % 3, (i + 2) % 3
            eng.tensor_tensor(out=tmps[0], in0=u[i1], in1=v[i2], op=OP.mult)
            eng.tensor_tensor(out=tmps[1], in0=u[i2], in1=v[i1], op=OP.mult)
            eng.tensor_tensor(out=dst[i], in0=tmps[0], in1=tmps[1],
                              op=OP.subtract)

    def qmul_into(nq, q1, q2, tmps):
        """nq = q1 (x) q2 elementwise (lists of 4 views). 28 DVE ops."""
        for comp in range(4):
            acc = nq[comp]
            V.tensor_tensor(out=acc, in0=q1[0], in1=q2[QIDX[comp][0]],
                            op=OP.mult)
            for t in range(1, 4):
                tmp = tmps[t % 2]
                V.tensor_tensor(out=tmp, in0=q1[t], in1=q2[QIDX[comp][t]],
                                op=OP.mult)
                V.tensor_tensor(out=acc, in0=acc, in1=tmp,
                                op=OP.add if QSGN[comp][t] > 0 else OP.subtract)

    with TileContext(nc) as tc:
        with (
            tc.tile_pool(name="slab", bufs=2) as slab_pool,
            tc.tile_pool(name="deint", bufs=1) as deint_pool,
            tc.tile_pool(name="plane", bufs=1) as plane_pool,
            tc.tile_pool(name="xtmp", bufs=2) as xtmp_pool,
            tc.tile_pool(name="acc", bufs=1) as acc_pool,
            tc.tile_pool(name="tree", bufs=1) as tree_pool,
        ):
            pshape = [128, g, 3, G]

            def plane(buf, nm):
                return plane_pool.tile(pshape, F32, tag=f"q{buf}", name=nm)

            def vtmp(n=None):
                return [xtmp_pool.tile([128, g, G], F32, tag="vt",
                                       name="vt")[:, :, 0:(n or G)]
                        for _ in range(2)]

            def gtmp(n=None):
                return [xtmp_pool.tile([128, g, G], F32, tag="gt",
                                       name="gt")[:, :, 0:(n or G)]
                        for _ in range(2)]

            # persistent accumulators: 16-step units across all slabs
            th16 = acc_pool.tile([128, g, 3, NU16], F32, tag="t16",
                                 name="t16")
            b16 = acc_pool.tile([128, g, 3, NU16], F32, tag="bb16",
                                name="bb16")
            r16 = acc_pool.tile([128, g, 3, NU16], F32, tag="r16", name="r16")

            def sa_fold(oth, cs, scal):
                # oth += scal * cs, per component (stt needs <=2 free dims)
                for ci in range(3):
                    V.scalar_tensor_tensor(out=oth[:, :, ci, :],
                                           in0=cs[:, :, ci, :], scalar=scal,
                                           in1=oth[:, :, ci, :],
                                           op0=OP.mult, op1=OP.add)

            def sa_level(thI, bI, rI, n_steps, oth, ob, orr, cs, cs2, tmp,
                         radd_eng):
                thAc = [thI[:, :, ci, 0::2] for ci in range(3)]
                thBc = [thI[:, :, ci, 1::2] for ci in range(3)]
                bBc = [bI[:, :, ci, 1::2] for ci in range(3)]
                cross_into(V, comps(cs), thAc, thBc, tmp)
                V.tensor_tensor(out=oth, in0=thI[:, :, :, 0::2],
                                in1=thI[:, :, :, 1::2], op=OP.add)
                sa_fold(oth, cs, DT / 2.0)
                cross_into(V, comps(cs2), thAc, bBc, tmp)
                V.tensor_tensor(out=ob, in0=bI[:, :, :, 0::2],
                                in1=bI[:, :, :, 1::2], op=OP.add)
                sa_fold(ob, cs2, DT)
                radd_eng.tensor_tensor(out=orr, in0=rI[:, :, :, 0::2],
                                       in1=rI[:, :, :, 1::2], op=OP.add)
                for ci in range(3):
                    V.scalar_tensor_tensor(out=orr[:, :, ci, :],
                                           in0=bI[:, :, ci, 0::2],
                                           scalar=float(n_steps),
                                           in1=orr[:, :, ci, :],
                                           op0=OP.mult, op1=OP.add)

            for si in range(n_slabs):
                slab = slab_pool.tile([128, g, s_chunk, C], F32, tag="slab",
                                      name=f"slab{si}")
                for gi in range(g):
                    nc.sync.dma_start(
                        out=slab[:, gi],
                        in_=xv[gi, :, si * s_chunk:(si + 1) * s_chunk, :],
                    )

                # ScalarE: deinterleave accel channels into dense tiles.
                # Order At1, At0 first: the DVE u-chain reads them first.
                At = [deint_pool.tile(pshape, F32, tag=f"at{i}",
                                      name=f"at{i}") for i in range(4)]
                for i in (1, 0, 2, 3):
                    for ci in range(3):
                        Sc.copy(At[i][:, :, ci, :], slab[:, :, i::4, 3 + ci])

                # DVE: w pair sums (strided slab reads), theta (dense),
                # p directly in bf16 (only feeds the bf16 correction cross)
                W = [slab[:, :, i::4, 0:3].transpose([0, 1, 3, 2])
                     for i in range(4)]
                s01 = plane(1, "s01")
                s23 = plane(2, "s23")
                th4 = plane(3, "th4")
                q4h = plane(4, "q4h")[:].bitcast(BF16)   # two bf16 planes
                p4h = q4h[:, :, :, 0:G]
                u4h = q4h[:, :, :, G:2 * G]
                q5h = plane(5, "q5h")[:].bitcast(BF16)
                th4h = q5h[:, :, :, 0:G]
                awh = q5h[:, :, :, G:2 * G]
                V.tensor_tensor(out=s01[:], in0=W[0], in1=W[1], op=OP.add)
                V.tensor_tensor(out=s23[:], in0=W[2], in1=W[3], op=OP.add)
                V.tensor_tensor(out=th4[:], in0=s01[:], in1=s23[:], op=OP.add)
                V.tensor_tensor(out=p4h, in0=s01[:], in1=s23[:],
                                op=OP.subtract)
                Sc.copy(th4h, th4[:])

                # DVE: BCH cross c3 = s01 x s23 in bf16 (cheap, no
                # cross-engine stall on the theta fold)
                c3h = plane_pool.tile([128, g, 3, G], BF16, tag="c3h",
                                      name="c3h")
                gt = [xtmp_pool.tile([128, g, G], F32, tag="gt",
                                     name="gt")[:].bitcast(BF16)[:, :, 0:G]
                      for _ in range(2)]
                cross_into(V, comps(c3h), comps(s01), comps(s23), gt)
                # GpSimd: V-chain over At (dense)
                vv = plane(8, "vv")
                Gp.tensor_tensor(out=vv[:], in0=At[0][:], in1=At[1][:],
                                 op=OP.add)
                Gp.tensor_tensor(out=vv[:], in0=vv[:], in1=At[2][:],
                                 op=OP.add)
                Gp.tensor_tensor(out=vv[:], in0=vv[:], in1=At[3][:],
                                 op=OP.add)

                # DVE: u-chain / aw / rho on dense At (flattened stt)
                t0 = plane(6, "t0")
                V.scalar_tensor_tensor(out=t0[:].opt(), in0=At[1][:].opt(),
                                       scalar=2.0, in1=At[0][:].opt(),
                                       op0=OP.mult, op1=OP.add)
                u4f = plane(7, "u4f")
                V.scalar_tensor_tensor(out=u4f[:].opt(), in0=At[2][:].opt(),
                                       scalar=3.0, in1=t0[:].opt(),
                                       op0=OP.mult, op1=OP.add)
                V.scalar_tensor_tensor(out=u4h.opt(), in0=At[3][:].opt(),
                                       scalar=4.0, in1=u4f[:].opt(),
                                       op0=OP.mult, op1=OP.add)
                # bf16 correction crosses (2x DVE mode)
                q2h = plane(2, "q2h")       # reuse s23 (dead after th/p)
                q2hb = q2h[:].bitcast(BF16)
                c1h = q2hb[:, :, :, 0:G]
                c2h = q2hb[:, :, :, G:2 * G]
                btmp = [xtmp_pool.tile([128, g, G], F32, tag="vt",
                                       name="vt")[:].bitcast(BF16)[:, :, 0:G]
                        for _ in range(2)]
                cross_into(V, comps(c1h), comps(th4h), comps(u4h), btmp)
                # aw = t0 + a2 (in place on t0)
                V.tensor_tensor(out=t0[:], in0=t0[:], in1=At[2][:], op=OP.add)
                aw = t0
                Sc.copy(awh, aw[:])
                # c2 = p4 x aw (bf16)
                cross_into(V, comps(c2h), comps(p4h), comps(awh), btmp)
                # rho = 2 a0 + aw  (u4f dead after its cast -> reuse Q7)
                rho4 = plane(7, "rho4")
                V.scalar_tensor_tensor(out=rho4[:].opt(), in0=At[0][:].opt(),
                                       scalar=2.0, in1=aw[:].opt(),
                                       op0=OP.mult, op1=OP.add)

                # folds: theta += (dt/2) c3 ; b = V + (dt/4)(c1+c2)
                V.scalar_tensor_tensor(out=th4[:].opt(), in0=c3h[:].opt(),
                                       scalar=DT / 2.0, in1=th4[:].opt(),
                                       op0=OP.mult, op1=OP.add)
                V.tensor_tensor(out=c1h, in0=c1h, in1=c2h, op=OP.add)
                V.scalar_tensor_tensor(out=vv[:].opt(), in0=c1h.opt(),
                                       scalar=DT / 4.0, in1=vv[:].opt(),
                                       op0=OP.mult, op1=OP.add)
                b4 = vv

                # ---- SA1: n=4 -> 8 ----
                th8 = plane(1, "th8")[:, :, :, 0:U8]   # s01 dead
                b8v = plane(2, "b8")[:, :, :, 0:U8]    # c1h/c2h dead
                r8v = plane(5, "r8")[:, :, :, 0:U8]    # th4h/awh dead
                csa = plane(4, "csa")[:, :, :, 0:U8]   # p4h/u4h dead
                csb = plane(6, "csb")[:, :, :, 0:U8]   # aw dead
                sa_level(th4[:], b4[:], rho4[:], 4, th8, b8v, r8v,
                         csa, csb, vtmp(U8), Gp)

                # ---- SA2: n=8 -> 16, into accumulators ----
                oth = th16[:, :, :, si * U16:(si + 1) * U16]
                ob = b16[:, :, :, si * U16:(si + 1) * U16]
                orr = r16[:, :, :, si * U16:(si + 1) * U16]
                csa2 = plane(4, "csa2")[:, :, :, 0:U16]
                csb2 = plane(6, "csb2")[:, :, :, 0:U16]
                sa_level(th8, b8v, r8v, 8, oth, ob, orr,
                         csa2, csb2, vtmp(U16), Gp)

            # ---- batched SA levels: 16->32->64 ----
            th32 = plane(1, "th32")[:, :, :, 0:64]
            b32 = plane(2, "b32")[:, :, :, 0:64]
            r32 = plane(5, "r32")[:, :, :, 0:64]
            cs_a = plane(4, "cs_a")[:, :, :, 0:64]
            cs_b = plane(6, "cs_b")[:, :, :, 0:64]
            sa_level(th16[:], b16[:], r16[:], 16, th32, b32, r32,
                     cs_a, cs_b, vtmp(64), Gp)
            th64 = plane(3, "th64")[:, :, :, 0:32]
            b64 = plane(8, "b64")[:, :, :, 0:32]
            r64 = plane(7, "r64")[:, :, :, 0:32]
            cs_a2 = plane(4, "cs_a2")[:, :, :, 0:32]
            cs_b2 = plane(6, "cs_b2")[:, :, :, 0:32]
            sa_level(th32, b32, r32, 32, th64, b64, r64,
                     cs_a2, cs_b2, vtmp(32), Gp)

            # ---- convert 64-step units to quaternions ----
            NU = 32
            K2 = (DT / 2.0) ** 2
            h2 = tree_pool.tile([128, g, NU], F32, tag="h2", name="h2")
            hy = tree_pool.tile([128, g, NU], F32, tag="hy", name="hy")
            hz = tree_pool.tile([128, g, NU], F32, tag="hz", name="hz")
            q64 = tree_pool.tile([128, g, 4, NU], F32, tag="q64", name="q64")
            V.tensor_tensor(out=h2[:], in0=th64[:, :, 0, :],
                            in1=th64[:, :, 0, :], op=OP.mult)
            for ci in (1, 2):
                V.tensor_tensor(out=hy[:], in0=th64[:, :, ci, :],
                                in1=th64[:, :, ci, :], op=OP.mult)
                V.tensor_tensor(out=h2[:], in0=h2[:], in1=hy[:], op=OP.add)
            # qw = 1 - (k/2) h2 + (k^2/24) h2^2
            V.scalar_tensor_tensor(out=hy[:], in0=h2[:],
                                   scalar=K2 * K2 / 24.0,
                                   in1=h2[:], op0=OP.mult, op1=OP.mult)
            V.scalar_tensor_tensor(out=hz[:], in0=h2[:], scalar=-K2 / 2.0,
                                   in1=hy[:], op0=OP.mult, op1=OP.add)
            Sc.activation(q64[:, :, 0, :], hz[:], AF.Identity, bias=1.0)
            # qv = (dt/2)(1 - (k/6) h2 + (k^2/120) h2^2) * theta
            V.scalar_tensor_tensor(out=hy[:], in0=h2[:],
                                   scalar=K2 * K2 / 120.0,
                                   in1=h2[:], op0=OP.mult, op1=OP.mult)
            V.scalar_tensor_tensor(out=hz[:], in0=h2[:], scalar=-K2 / 6.0,
                                   in1=hy[:], op0=OP.mult, op1=OP.add)
            Sc.activation(hz[:], hz[:], AF.Copy, scale=DT / 2.0,
                          bias=DT / 2.0)
            for ci in range(3):
                V.tensor_tensor(out=q64[:, :, 1 + ci, :],
                                in0=th64[:, :, ci, :], in1=hz[:], op=OP.mult)

            # ---- quaternion binary tree over 32 units (5 levels) ----
            cur_q = [q64[:, :, ci, :] for ci in range(4)]
            cur_b = [b64[:, :, ci, :] for ci in range(3)]
            cur_d = [r64[:, :, ci, :] for ci in range(3)]
            n2 = 64.0
            n_lvl = int(math.log2(NU))
            for lvl in range(1, n_lvl + 1):
                n = NU >> lvl
                lshp = [128, g, n]
                q1 = [t[:, :, 0::2] for t in cur_q]
                q2 = [t[:, :, 1::2] for t in cur_q]
                b1 = [t[:, :, 0::2] for t in cur_b]
                b2 = [t[:, :, 1::2] for t in cur_b]
                d1 = [t[:, :, 0::2] for t in cur_d]
                d2 = [t[:, :, 1::2] for t in cur_d]

                nq = [tree_pool.tile(lshp, F32, tag=f"tq{lvl}_{i}",
                                     name=f"tq{lvl}_{i}")[:] for i in range(4)]
                nb = [tree_pool.tile(lshp, F32, tag=f"tb{lvl}_{i}",
                                     name=f"tb{lvl}_{i}")[:] for i in range(3)]
                nd = [tree_pool.tile(lshp, F32, tag=f"td{lvl}_{i}",
                                     name=f"td{lvl}_{i}")[:] for i in range(3)]
                tc_c = [tree_pool.tile(lshp, F32, tag=f"tc{i}",
                                       name=f"tc{i}")[:] for i in range(3)]
                tc_w = [tree_pool.tile(lshp, F32, tag=f"tw{i}",
                                       name=f"tw{i}")[:] for i in range(3)]
                tm = [tree_pool.tile(lshp, F32, tag=f"tm{i}",
                                     name=f"tm{i}")[:] for i in range(2)]

                qmul_into(nq, q1, q2, tm)
                qv1 = q1[1:]

                # nd = d1 + n2*b1 + rot(q1, d2)
                cross_into(V, tc_c, qv1, d2, tm)
                for i in range(3):
                    V.tensor_tensor(out=tc_w[i], in0=q1[0], in1=d2[i],
                                    op=OP.mult)
                for i in range(3):
                    V.tensor_tensor(out=tc_c[i], in0=tc_c[i], in1=tc_w[i],
                                    op=OP.add)
                cross_into(V, tc_w, qv1, tc_c, tm)
                for i in range(3):
                    V.scalar_tensor_tensor(out=nd[i], in0=b1[i], scalar=n2,
                                           in1=d1[i], op0=OP.mult, op1=OP.add)
                for i in range(3):
                    Gp.tensor_tensor(out=nd[i], in0=nd[i], in1=d2[i],
                                     op=OP.add)
                for i in range(3):
                    V.scalar_tensor_tensor(out=nd[i], in0=tc_w[i], scalar=2.0,
                                           in1=nd[i], op0=OP.mult, op1=OP.add)

                # nb = b1 + rot(q1, b2)
                cross_into(V, tc_c, qv1, b2, tm)
                for i in range(3):
                    V.tensor_tensor(out=tc_w[i], in0=q1[0], in1=b2[i],
                                    op=OP.mult)
                for i in range(3):
                    V.tensor_tensor(out=tc_c[i], in0=tc_c[i], in1=tc_w[i],
                                    op=OP.add)
                cross_into(V, tc_w, qv1, tc_c, tm)
                for i in range(3):
                    Gp.tensor_tensor(out=nb[i], in0=b1[i], in1=b2[i],
                                     op=OP.add)
                for i in range(3):
                    V.scalar_tensor_tensor(out=nb[i], in0=tc_w[i], scalar=2.0,
                                           in1=nb[i], op0=OP.mult, op1=OP.add)

                cur_q, cur_b, cur_d = nq, nb, nd
                n2 *= 2.0

            # ---- finalize ----
            out_t = tree_pool.tile([128, g, 7], F32, tag="outt",
                                   name="outt")[:]
            tf = tree_pool.tile([128, g, 1], F32, tag="tf", name="tf")[:]
            for i in range(3):
                V.scalar_tensor_tensor(out=tf, in0=cur_b[i], scalar=1.5,
                                       in1=cur_d[i], op0=OP.mult, op1=OP.add)
                V.tensor_scalar(out=out_t[:, :, i:i + 1], in0=tf,
                                scalar1=DT * DT, scalar2=None, op0=OP.mult)
            sg = tree_pool.tile([128, g, 1], F32, tag="sg", name="sg")[:]
            V.tensor_scalar(out=sg, in0=cur_q[0], scalar1=0.0, scalar2=2.0,
                            op0=OP.is_ge, op1=OP.mult)
            V.tensor_scalar(out=sg, in0=sg, scalar1=-1.0, scalar2=None,
                            op0=OP.add)
            for i in range(4):
                V.tensor_tensor(out=out_t[:, :, 3 + i:4 + i], in0=cur_q[i],
                                in1=sg, op=OP.mult)

            ov = out.rearrange("(g p) c -> g p c", g=g)
            for gi in range(g):
                nc.sync.dma_start(out=ov[gi], in_=out_t[:, gi, :])

    nc.compile()
    return nc


_NC_CACHE = {}
LAST_RESULTS = None


def _ensure_profiling_hooks():
    """Best-effort: provide the antenv.axon_hooks shim + skip S3 upload so
    trace=True works in this stripped container. No-op on failure."""
    import sys
    import types
    try:
        if "antenv.axon_hooks" not in sys.modules:
            from trn_agent_boot.trn_boot import _ntff_profile_via_ctypes
            hook = _ntff_profile_via_ctypes("/opt/axon/libaxon_pjrt.so")
            mod = types.ModuleType("antenv.axon_hooks")
            mod._hook = hook
            mod.get_axon_ntff_profile_hook = lambda: mod._hook
            mod.set_axon_ntff_profile_hook = lambda h: setattr(mod, "_hook", h)
            sys.modules["antenv.axon_hooks"] = mod
        import concourse.bass_utils as bu
        bu.upload_artifacts = lambda tmpdir: tmpdir
    except Exception as e:  # pragma: no cover
        print(f"profiling hook setup failed ({e}); tracing may be skipped")


def kernel(input_seq: np.ndarray) -> np.ndarray:
    from concourse.bass_utils import run_bass_kernel_spmd

    global LAST_RESULTS
    input_seq = np.ascontiguousarray(np.asarray(input_seq, dtype=np.float32))
    assert input_seq.shape == (B_FULL, S_FULL, C), input_seq.shape

    if "nc" not in _NC_CACHE:
        _NC_CACHE["nc"] = build_nc()
    nc = _NC_CACHE["nc"]

    in_maps = [{"x": input_seq[i * R:(i + 1) * R]} for i in range(N_CORES)]
    trace = os.environ.get("BASS_KERNEL_TRACE", "0") == "1"
    if trace:
        _ensure_profiling_hooks()
    res = run_bass_kernel_spmd(nc, in_maps, core_ids=list(range(N_CORES)),
                               trace=trace)
    LAST_RESULTS = res
    return np.concatenate([r["out"] for r in res.results], axis=0)


# revision 16
# speedup vs baseline: 1.0436x; 1.0009x over previous
"""Trainium2 Bass kernel for the IMU preintegration module.

Full inputs in, full outputs out; internally data-parallel over 8 NeuronCores
(512 batch rows per core).

Math: the scan's per-row state composes associatively as (R, b, d) with
    b = sum_t R_{1..t} a_t,   d = sum_t (S-1-t) R_{1..t} a_t   (raw units;
physical factors of dt are folded into scalars).  Composition of two units
A,B of n steps each:
    R = RA RB,  b = bA + RA bB,  d = dA + n bA + RA dB.
Per-step rotations are tiny (|w| dt ~ 0.01 rad), so:
  L0: groups of n0=4 steps reduce to (theta, b, rho) with first-order
      rotation:  theta = sum w (+ BCH pair term), b = V + (dt/4)(theta x u
      + p x aw)  [u = ramp-weighted a-sum, p = Haar mode of w, aw its
      weight], rho = 3a0+2a1+a2.
  SA: small-angle composition up to 64-step units:
      theta' = tA+tB + (dt/2) tA x tB,  b' = bA+bB + dt (tA x bB),
      rho' = rA + n bA + rB.
  QT: convert theta -> quaternion (2-3 term polys in h = |theta dt/2|^2),
      exact quaternion binary tree for the remaining 5 levels.
Validated in numpy against the jax reference: rel err ~3.7e-3 (gate 2e-2).

Perf notes (hardware-measured): DVE fp32 ops run 1 elem/cycle at read
stride <= 8B, ~1.5x slower at 16-48B, ~2x at 96B.  So ScalarE (otherwise
idle) deinterleaves the accel channels into dense tiles; DVE reads only
dense / stride-2 data except the two w pair-sum ops.  GpSimd takes the
independent V-chain + one cross per slab.  SBUF: 8 rotating 6KB plane
buffers (Q1-Q8) hold all group-level 3-vectors via explicit reuse.
"""

import math
import os
import numpy as np

import concourse.mybir as mybir
from concourse import bass, bacc
from concourse.tile import TileContext

F32 = mybir.dt.float32
BF16 = mybir.dt.bfloat16
OP = mybir.AluOpType
AF = mybir.ActivationFunctionType

# problem constants (hardcoded per harness contract)
B_FULL = 4096
S_FULL = 2048
C = 6
N_CORES = 8
R = B_FULL // N_CORES          # rows per core = 512
DT = float(np.float32(1.0 / 200.0))

QSGN = [(1, -1, -1, -1), (1, 1, 1, -1), (1, -1, 1, 1), (1, 1, -1, 1)]
QIDX = [(0, 1, 2, 3), (1, 0, 3, 2), (2, 3, 0, 1), (3, 2, 1, 0)]


def build_nc(rows=R, s_len=S_FULL, s_chunk=512):
    nc = bacc.Bacc(None, target_bir_lowering=False, debug=False)
    g = rows // 128                    # 4 rows per partition
    n_slabs = s_len // s_chunk         # 4
    G = s_chunk // 4                   # 128 4-step groups per slab
    U8 = s_chunk // 8                  # 64 8-step units per slab
    U16 = s_chunk // 16                # 32 16-step units per slab
    NU16 = s_len // 16                 # 128 16-step units per row
    x = nc.dram_tensor("x", [rows, s_len, C], F32, kind="ExternalInput")
    out = nc.dram_tensor("out", [rows, 7], F32, kind="ExternalOutput")
    xv = x.rearrange("(g p) s c -> g p s c", g=g)

    V = nc.vector
    Gp = nc.gpsimd
    Sc = nc.scalar

    def comps(t, n=3):
        return [t[:, :, ci, :] for ci in range(n)]

    def cross_into(eng, dst, u, v, tmps):
        """dst = u x v per-component (lists of [128,g,U] views). 9 ops."""
        for i in range(3):
            i1, i2 = (i + 1) % 3, (i + 2) % 3
            eng.tensor_tensor(out=tmps[0], in0=u[i1], in1=v[i2], op=OP.mult)
            eng.tensor_tensor(out=tmps[1], in0=u[i2], in1=v[i1], op=OP.mult)
            eng.tensor_tensor(out=dst[i], in0=tmps[0], in1=tmps[1],
                              op=OP.subtract)

    def qmul_into(nq, q1, q2, tmps):
        """nq = q1 (x) q2 elementwise (lists of 4 views). 28 DVE ops."""
        for comp in range(4):
            acc = nq[comp]
            V.tensor_tensor(out=acc, in0=q1[0], in1=q2[QIDX[comp][0]],
                            op=OP.mult)
            for t in range(1, 4):
                tmp = tmps[t % 2]
                V.tensor_tensor(out=tmp, in0=q1[t], in1=q2[QIDX[comp][t]],
                                op=OP.mult)
                V.tensor_tensor(out=acc, in0=acc, in1=tmp,
                                op=OP.add if QSGN[comp][t] > 0 else OP.subtract)

    with TileContext(nc) as tc:
        with (
            tc.tile_pool(name="slab", bufs=2) as slab_pool,
            tc.tile_pool(name="deint", bufs=1) as deint_pool,
            tc.tile_pool(name="plane", bufs=1) as plane_pool,
            tc.tile_pool(name="xtmp", bufs=2) as xtmp_pool,
            tc.tile_pool(name="acc", bufs=1) as acc_pool,
            tc.tile_pool(name="tree", bufs=1) as tree_pool,
        ):
            pshape = [128, g, 3, G]

            def plane(buf, nm):
                return plane_pool.tile(pshape, F32, tag=f"q{buf}", name=nm)

            def vtmp(n=None):
                return [xtmp_pool.tile([128, g, G], F32, tag="vt",
                                       name="vt")[:, :, 0:(n or G)]
                        for _ in range(2)]

            def gtmp(n=None):
                return [xtmp_pool.tile([128, g, G], F32, tag="gt",
                                       name="gt")[:, :, 0:(n or G)]
                        for _ in range(2)]

            # persistent accumulators: 16-step units across all slabs
            th16 = acc_pool.tile([128, g, 3, NU16], F32, tag="t16",
                                 name="t16")
            b16 = acc_pool.tile([128, g, 3, NU16], F32, tag="bb16",
                                name="bb16")
            r16 = acc_pool.tile([128, g, 3, NU16], F32, tag="r16", name="r16")

            def sa_fold(oth, cs, scal):
                # oth += scal * cs, per component (stt needs <=2 free dims)
                for ci in range(3):
                    V.scalar_tensor_tensor(out=oth[:, :, ci, :],
                                           in0=cs[:, :, ci, :], scalar=scal,
                                           in1=oth[:, :, ci, :],
                                           op0=OP.mult, op1=OP.add)

            def sa_level(thI, bI, rI, n_steps, oth, ob, orr, cs, cs2, tmp,
                         radd_eng):
                thAc = [thI[:, :, ci, 0::2] for ci in range(3)]
                thBc = [thI[:, :, ci, 1::2] for ci in range(3)]
                bBc = [bI[:, :, ci, 1::2] for ci in range(3)]
                cross_into(V, comps(cs), thAc, thBc, tmp)
                V.tensor_tensor(out=oth, in0=thI[:, :, :, 0::2],
                                in1=thI[:, :, :, 1::2], op=OP.add)
                sa_fold(oth, cs, DT / 2.0)
                cross_into(V, comps(cs2), thAc, bBc, tmp)
                V.tensor_tensor(out=ob, in0=bI[:, :, :, 0::2],
                                in1=bI[:, :, :, 1::2], op=OP.add)
                sa_fold(ob, cs2, DT)
                radd_eng.tensor_tensor(out=orr, in0=rI[:, :, :, 0::2],
                                       in1=rI[:, :, :, 1::2], op=OP.add)
                for ci in range(3):
                    V.scalar_tensor_tensor(out=orr[:, :, ci, :],
                                           in0=bI[:, :, ci, 0::2],
                                           scalar=float(n_steps),
                                           in1=orr[:, :, ci, :],
                                           op0=OP.mult, op1=OP.add)

            for si in range(n_slabs):
                slab = slab_pool.tile([128, g, s_chunk, C], F32, tag="slab",
                                      name=f"slab{si}")
                for gi in range(g):
                    nc.sync.dma_start(
                        out=slab[:, gi],
                        in_=xv[gi, :, si * s_chunk:(si + 1) * s_chunk, :],
                    )

                # ScalarE: deinterleave accel channels into dense tiles.
                # Order At1, At0 first: the DVE u-chain reads them first.
                At = [deint_pool.tile(pshape, F32, tag=f"at{i}",
                                      name=f"at{i}") for i in range(4)]
                for i in (1, 0, 2, 3):
                    for ci in range(3):
                        Sc.copy(At[i][:, :, ci, :], slab[:, :, i::4, 3 + ci])

                # DVE: w pair sums (strided slab reads), theta (dense),
                # p directly in bf16 (only feeds the bf16 correction cross)
                W = [slab[:, :, i::4, 0:3].transpose([0, 1, 3, 2])
                     for i in range(4)]
                s01 = plane(1, "s01")
                s23 = plane(2, "s23")
                th4 = plane(3, "th4")
                q4h = plane(4, "q4h")[:].bitcast(BF16)   # two bf16 planes
                p4h = q4h[:, :, :, 0:G]
                u4h = q4h[:, :, :, G:2 * G]
                q5h = plane(5, "q5h")[:].bitcast(BF16)
                th4h = q5h[:, :, :, 0:G]
                awh = q5h[:, :, :, G:2 * G]
                V.tensor_tensor(out=s01[:], in0=W[0], in1=W[1], op=OP.add)
                V.tensor_tensor(out=s23[:], in0=W[2], in1=W[3], op=OP.add)
                V.tensor_tensor(out=th4[:], in0=s01[:], in1=s23[:], op=OP.add)
                V.tensor_tensor(out=p4h, in0=s01[:], in1=s23[:],
                                op=OP.subtract)
                Sc.copy(th4h, th4[:])

                # DVE: BCH cross c3 = s01 x s23 in bf16 (cheap, no
                # cross-engine stall on the theta fold)
                c3h = plane_pool.tile([128, g, 3, G], BF16, tag="c3h",
                                      name="c3h")
                gt = [xtmp_pool.tile([128, g, G], F32, tag="gt",
                                     name="gt")[:].bitcast(BF16)[:, :, 0:G]
                      for _ in range(2)]
                cross_into(V, comps(c3h), comps(s01), comps(s23), gt)
                # GpSimd: V-chain over At (dense)
                vv = plane(8, "vv")
                Gp.tensor_tensor(out=vv[:], in0=At[0][:], in1=At[1][:],
                                 op=OP.add)
                Gp.tensor_tensor(out=vv[:], in0=vv[:], in1=At[2][:],
                                 op=OP.add)
                Gp.tensor_tensor(out=vv[:], in0=vv[:], in1=At[3][:],
                                 op=OP.add)

                # DVE: u-chain / aw / rho on dense At (flattened stt)
                t0 = plane(6, "t0")
                V.scalar_tensor_tensor(out=t0[:].opt(), in0=At[1][:].opt(),
                                       scalar=2.0, in1=At[0][:].opt(),
                                       op0=OP.mult, op1=OP.add)
                u4f = plane(7, "u4f")
                V.scalar_tensor_tensor(out=u4f[:].opt(), in0=At[2][:].opt(),
                                       scalar=3.0, in1=t0[:].opt(),
                                       op0=OP.mult, op1=OP.add)
                V.scalar_tensor_tensor(out=u4h.opt(), in0=At[3][:].opt(),
                                       scalar=4.0, in1=u4f[:].opt(),
                                       op0=OP.mult, op1=OP.add)
                # bf16 correction crosses (2x DVE mode)
                q2h = plane(2, "q2h")       # reuse s23 (dead after th/p)
                q2hb = q2h[:].bitcast(BF16)
                c1h = q2hb[:, :, :, 0:G]
                c2h = q2hb[:, :, :, G:2 * G]
                btmp = [xtmp_pool.tile([128, g, G], F32, tag="vt",
                                       name="vt")[:].bitcast(BF16)[:, :, 0:G]
                        for _ in range(2)]
                cross_into(V, comps(c1h), comps(th4h), comps(u4h), btmp)
                # aw = t0 + a2 (in place on t0)
                V.tensor_tensor(out=t0[:], in0=t0[:], in1=At[2][:], op=OP.add)
                aw = t0
                Sc.copy(awh, aw[:])
                # c2 = p4 x aw (bf16)
                cross_into(V, comps(c2h), comps(p4h), comps(awh), btmp)
                # rho = 2 a0 + aw  (u4f dead after its cast -> reuse Q7)
                rho4 = plane(7, "rho4")
                V.scalar_tensor_tensor(out=rho4[:].opt(), in0=At[0][:].opt(),
                                       scalar=2.0, in1=aw[:].opt(),
                                       op0=OP.mult, op1=OP.add)

                # folds: theta += (dt/2) c3 ; b = V + (dt/4)(c1+c2)
                V.scalar_tensor_tensor(out=th4[:].opt(), in0=c3h[:].opt(),
                                       scalar=DT / 2.0, in1=th4[:].opt(),
                                       op0=OP.mult, op1=OP.add)
                V.tensor_tensor(out=c1h, in0=c1h, in1=c2h, op=OP.add)
                V.scalar_tensor_tensor(out=vv[:].opt(), in0=c1h.opt(),
                                       scalar=DT / 4.0, in1=vv[:].opt(),
                                       op0=OP.mult, op1=OP.add)
                b4 = vv

                # ---- SA1: n=4 -> 8 ----
                th8 = plane(1, "th8")[:, :, :, 0:U8]   # s01 dead
                b8v = plane(2, "b8")[:, :, :, 0:U8]    # c1h/c2h dead
                r8v = plane(5, "r8")[:, :, :, 0:U8]    # th4h/awh dead
                csa = plane(4, "csa")[:, :, :, 0:U8]   # p4h/u4h dead
                csb = plane(6, "csb")[:, :, :, 0:U8]   # aw dead
                sa_level(th4[:], b4[:], rho4[:], 4, th8, b8v, r8v,
                         csa, csb, vtmp(U8), Gp)

                # ---- SA2: n=8 -> 16, into accumulators ----
                oth = th16[:, :, :, si * U16:(si + 1) * U16]
                ob = b16[:, :, :, si * U16:(si + 1) * U16]
                orr = r16[:, :, :, si * U16:(si + 1) * U16]
                csa2 = plane(4, "csa2")[:, :, :, 0:U16]
                csb2 = plane(6, "csb2")[:, :, :, 0:U16]
                sa_level(th8, b8v, r8v, 8, oth, ob, orr,
                         csa2, csb2, vtmp(U16), Gp)

            # ---- batched SA levels: 16->32->64 ----
            th32 = plane(1, "th32")[:, :, :, 0:64]
            b32 = plane(2, "b32")[:, :, :, 0:64]
            r32 = plane(5, "r32")[:, :, :, 0:64]
            cs_a = plane(4, "cs_a")[:, :, :, 0:64]
            cs_b = plane(6, "cs_b")[:, :, :, 0:64]
            sa_level(th16[:], b16[:], r16[:], 16, th32, b32, r32,
                     cs_a, cs_b, vtmp(64), Gp)
            th64 = plane(3, "th64")[:, :, :, 0:32]
            b64 = plane(8, "b64")[:, :, :, 0:32]
            r64 = plane(7, "r64")[:, :, :, 0:32]
            cs_a2 = plane(4, "cs_a2")[:, :, :, 0:32]
            cs_b2 = plane(6, "cs_b2")[:, :, :, 0:32]
            sa_level(th32, b32, r32, 32, th64, b64, r64,
                     cs_a2, cs_b2, vtmp(32), Gp)

            # ---- convert 64-step units to quaternions ----
            NU = 32
            K2 = (DT / 2.0) ** 2
            h2 = tree_pool.tile([128, g, NU], F32, tag="h2", name="h2")
            hy = tree_pool.tile([128, g, NU], F32, tag="hy", name="hy")
            hz = tree_pool.tile([128, g, NU], F32, tag="hz", name="hz")
            q64 = tree_pool.tile([128, g, 4, NU], F32, tag="q64", name="q64")
            V.tensor_tensor(out=h2[:], in0=th64[:, :, 0, :],
                            in1=th64[:, :, 0, :], op=OP.mult)
            for ci in (1, 2):
                V.tensor_tensor(out=hy[:], in0=th64[:, :, ci, :],
                                in1=th64[:, :, ci, :], op=OP.mult)
                V.tensor_tensor(out=h2[:], in0=h2[:], in1=hy[:], op=OP.add)
            # qw = 1 - (k/2) h2 + (k^2/24) h2^2
            V.scalar_tensor_tensor(out=hy[:], in0=h2[:],
                                   scalar=K2 * K2 / 24.0,
                                   in1=h2[:], op0=OP.mult, op1=OP.mult)
            V.scalar_tensor_tensor(out=hz[:], in0=h2[:], scalar=-K2 / 2.0,
                                   in1=hy[:], op0=OP.mult, op1=OP.add)
            Sc.activation(q64[:, :, 0, :], hz[:], AF.Identity, bias=1.0)
            # qv = (dt/2)(1 - (k/6) h2 + (k^2/120) h2^2) * theta
            V.scalar_tensor_tensor(out=hy[:], in0=h2[:],
                                   scalar=K2 * K2 / 120.0,
                                   in1=h2[:], op0=OP.mult, op1=OP.mult)
            V.scalar_tensor_tensor(out=hz[:], in0=h2[:], scalar=-K2 / 6.0,
                                   in1=hy[:], op0=OP.mult, op1=OP.add)
            Sc.activation(hz[:], hz[:], AF.Copy, scale=DT / 2.0,
                          bias=DT / 2.0)
            for ci in range(3):
                V.tensor_tensor(out=q64[:, :, 1 + ci, :],
                                in0=th64[:, :, ci, :], in1=hz[:], op=OP.mult)

            # ---- quaternion binary tree over 32 units (5 levels) ----
            cur_q = [q64[:, :, ci, :] for ci in range(4)]
            cur_b = [b64[:, :, ci, :] for ci in range(3)]
            cur_d = [r64[:, :, ci, :] for ci in range(3)]
            n2 = 64.0
            n_lvl = int(math.log2(NU))
            for lvl in range(1, n_lvl + 1):
                n = NU >> lvl
                lshp = [128, g, n]
                q1 = [t[:, :, 0::2] for t in cur_q]
                q2 = [t[:, :, 1::2] for t in cur_q]
                b1 = [t[:, :, 0::2] for t in cur_b]
                b2 = [t[:, :, 1::2] for t in cur_b]
                d1 = [t[:, :, 0::2] for t in cur_d]
                d2 = [t[:, :, 1::2] for t in cur_d]

                nq = [tree_pool.tile(lshp, F32, tag=f"tq{lvl}_{i}",
                                     name=f"tq{lvl}_{i}")[:] for i in range(4)]
                nb = [tree_pool.tile(lshp, F32, tag=f"tb{lvl}_{i}",
                                     name=f"tb{lvl}_{i}")[:] for i in range(3)]
                nd = [tree_pool.tile(lshp, F32, tag=f"td{lvl}_{i}",
                                     name=f"td{lvl}_{i}")[:] for i in range(3)]
                tc_c = [tree_pool.tile(lshp, F32, tag=f"tc{i}",
                                       name=f"tc{i}")[:] for i in range(3)]
                tc_w = [tree_pool.tile(lshp, F32, tag=f"tw{i}",
                                       name=f"tw{i}")[:] for i in range(3)]
                tm = [tree_pool.tile(lshp, F32, tag=f"tm{i}",
                                     name=f"tm{i}")[:] for i in range(2)]

                qmul_into(nq, q1, q2, tm)
                qv1 = q1[1:]

                # nd = d1 + n2*b1 + rot(q1, d2)
                cross_into(V, tc_c, qv1, d2, tm)
                for i in range(3):
                    V.tensor_tensor(out=tc_w[i], in0=q1[0], in1=d2[i],
                                    op=OP.mult)
                for i in range(3):
                    V.tensor_tensor(out=tc_c[i], in0=tc_c[i], in1=tc_w[i],
                                    op=OP.add)
                cross_into(V, tc_w, qv1, tc_c, tm)
                for i in range(3):
                    V.scalar_tensor_tensor(out=nd[i], in0=b1[i], scalar=n2,
                                           in1=d1[i], op0=OP.mult, op1=OP.add)
                for i in range(3):
                    Gp.tensor_tensor(out=nd[i], in0=nd[i], in1=d2[i],
                                     op=OP.add)
                for i in range(3):
                    V.scalar_tensor_tensor(out=nd[i], in0=tc_w[i], scalar=2.0,
                                           in1=nd[i], op0=OP.mult, op1=OP.add)

                # nb = b1 + rot(q1, b2)
                cross_into(V, tc_c, qv1, b2, tm)
                for i in range(3):
                    V.tensor_tensor(out=tc_w[i], in0=q1[0], in1=b2[i],
                                    op=OP.mult)
                for i in range(3):
                    V.tensor_tensor(out=tc_c[i], in0=tc_c[i], in1=tc_w[i],
                                    op=OP.add)
                cross_into(V, tc_w, qv1, tc_c, tm)
                for i in range(3):
                    Gp.tensor_tensor(out=nb[i], in0=b1[i], in1=b2[i],
                                     op=OP.add)
                for i in range(3):
                    V.scalar_tensor_tensor(out=nb[i], in0=tc_w[i], scalar=2.0,
                                           in1=nb[i], op0=OP.mult, op1=OP.add)

                cur_q, cur_b, cur_d = nq, nb, nd
                n2 *= 2.0

            # ---- finalize ----
            out_t = tree_pool.tile([128, g, 7], F32, tag="outt",
                                   name="outt")[:]
            tf = tree_pool.tile([128, g, 1], F32, tag="tf", name="tf")[:]
            for i in range(3):
                V.scalar_tensor_tensor(out=tf, in0=cur_b[i], scalar=1.5,
                                       in1=cur_d[i], op0=OP.mult, op1=OP.add)
                V.tensor_scalar(out=out_t[:, :, i:i + 1], in0=tf,
                                scalar1=DT * DT, scalar2=None, op0=OP.mult)
            sg = tree_pool.tile([128, g, 1], F32, tag="sg", name="sg")[:]
            V.tensor_scalar(out=sg, in0=cur_q[0], scalar1=0.0, scalar2=2.0,
                            op0=OP.is_ge, op1=OP.mult)
            V.tensor_scalar(out=sg, in0=sg, scalar1=-1.0, scalar2=None,
                            op0=OP.add)
            for i in range(4):
                V.tensor_tensor(out=out_t[:, :, 3 + i:4 + i], in0=cur_q[i],
                                in1=sg, op=OP.mult)

            ov = out.rearrange("(g p) c -> g p c", g=g)
            for gi in range(g):
                nc.sync.dma_start(out=ov[gi], in_=out_t[:, gi, :])

    nc.compile()
    return nc


_NC_CACHE = {}
LAST_RESULTS = None


def _ensure_profiling_hooks():
    """Best-effort: provide the antenv.axon_hooks shim + skip S3 upload so
    trace=True works in this stripped container. No-op on failure."""
    import sys
    import types
    try:
        if "antenv.axon_hooks" not in sys.modules:
            from trn_agent_boot.trn_boot import _ntff_profile_via_ctypes
            hook = _ntff_profile_via_ctypes("/opt/axon/libaxon_pjrt.so")
            mod = types.ModuleType("antenv.axon_hooks")
            mod._hook = hook
            mod.get_axon_ntff_profile_hook = lambda: mod._hook
            mod.set_axon_ntff_profile_hook = lambda h: setattr(mod, "_hook", h)
            sys.modules["antenv.axon_hooks"] = mod
        import concourse.bass_utils as bu
        bu.upload_artifacts = lambda tmpdir: tmpdir
    except Exception as e:  # pragma: no cover
        print(f"profiling hook setup failed ({e}); tracing may be skipped")


def kernel(input_seq: np.ndarray) -> np.ndarray:
    from concourse.bass_utils import run_bass_kernel_spmd

    global LAST_RESULTS
    input_seq = np.ascontiguousarray(np.asarray(input_seq, dtype=np.float32))
    assert input_seq.shape == (B_FULL, S_FULL, C), input_seq.shape

    if "nc" not in _NC_CACHE:
        _NC_CACHE["nc"] = build_nc()
    nc = _NC_CACHE["nc"]

    in_maps = [{"x": input_seq[i * R:(i + 1) * R]} for i in range(N_CORES)]
    trace = os.environ.get("BASS_KERNEL_TRACE", "0") == "1"
    if trace:
        _ensure_profiling_hooks()
    try:
        res = run_bass_kernel_spmd(nc, in_maps, core_ids=list(range(N_CORES)),
                                   trace=trace)
    except Exception:
        # transient device wedge (NRT_EXEC_UNIT_UNRECOVERABLE) recovers on
        # a clean re-run; retry once
        res = run_bass_kernel_spmd(nc, in_maps, core_ids=list(range(N_CORES)),
                                   trace=trace)
    LAST_RESULTS = res
    return np.concatenate([r["out"] for r in res.results], axis=0)
